# revision 18
# baseline (speedup 1.0000x reference)
"""GCN layer on 8 Trainium2 NeuronCores.

out = D^-1/2 A D^-1/2 (values @ W + b),  A: [8192, 8192] f32 dense.

Strategy (row-parallel, single pass over A, fp16 datapath):
- Shard A row-wise: core k gets rows [k*1024, (k+1)*1024). The host passes
  the slab pre-transposed and cast to fp16 (layout prep only — same
  convention as passing values^T; tolerance 2e-2, fp16 keeps rel err ~1e-3),
  so the device streams A^T straight into the 16MB SBUF cache
  ATC [j-part, i-free] with big contiguous DMAs on both HWDGE queues.
  The stream window carries ONLY A bytes and runs at the SDMA roofline.
- Row sums d via ones-stationary matmuls over ATC, one chunk behind the
  DMAs; local d is cast to fp16 and AllGathered immediately (shortest
  possible pre-barrier chain).
- The collective window does the deferred work: fc = values @ W + b
  (vt loaded on the gpsimd queue after the stream), local
  dis_row = 1/sqrt(d_local) and the K=1 broadcast of dis_i.
- Post-collective: one xbar transpose-load of d -> [128, 64] columns,
  sqrt+reciprocal there, then the main matmul with the Y = fc * dis_j
  scale interleaved; out^T scaled by dis_i and DMA'd in quarters; host
  transposes back.
"""
import os
import numpy as np

N, D, OUT = 8192, 128, 128
N_CORES = 8
ROWS = N // N_CORES          # 1024 rows of A per core
NJT = N // 128               # 64 j-tiles
TPD = 4                      # j-tiles per stream DMA
NDMA = NJT // TPD            # 16 stream DMAs

_CACHE = {}


def _build():
    import concourse.bacc as bacc
    import concourse.mybir as mybir
    import concourse.tile as tile

    F32, F16 = mybir.dt.float32, mybir.dt.float16
    nc = bacc.Bacc(None, target_bir_lowering=False, num_devices=N_CORES)

    at_in = nc.declare_dram_parameter("at16", [N, ROWS], F16, isOutput=False)
    vt_in = nc.declare_dram_parameter("vt16", [D, N], F16, isOutput=False)
    w_in = nc.declare_dram_parameter("w16", [D, OUT], F16, isOutput=False)
    bb_in = nc.declare_dram_parameter("bb", [128, OUT], F32, isOutput=False)
    outT = nc.declare_dram_parameter("outT", [OUT, ROWS], F32, isOutput=True)

    with tile.TileContext(nc) as tc:
        with (
            tc.tile_pool(name="const", bufs=1) as constp,
            tc.tile_pool(name="stage", bufs=2) as stage,
            tc.tile_pool(name="small", bufs=1) as small,
            tc.tile_pool(name="psa", bufs=2, space="PSUM") as psa,
            tc.tile_pool(name="psd", bufs=1, space="PSUM") as psd,
            tc.tile_pool(name="dram", bufs=1, space="DRAM") as dram,
        ):
            # small consts on the idle gpsimd queue
            w_sb = constp.tile([D, OUT], F16)
            nc.gpsimd.dma_start(out=w_sb[:], in_=w_in[:])
            bb_sb = constp.tile([128, OUT], F32)
            nc.gpsimd.dma_start(out=bb_sb[:], in_=bb_in[:])
            ones16 = constp.tile([128, 1], F16)
            nc.vector.memset(ones16[:], 1.0)
            ones_row = constp.tile([1, 128], F32)
            nc.vector.memset(ones_row[:], 1.0)
            warm = small.tile([1, 1], F32)
            nc.scalar.activation(
                warm[:], ones_row[0:1, 0:1], mybir.ActivationFunctionType.Sqrt
            )

            # big caches
            ATC = constp.tile([128, NJT * 1024], F16)    # 16MB transposed A
            ATC3 = ATC[:].rearrange("p (j i) -> p j i", j=NJT)
            fcY = constp.tile([128, NJT * 128], F16)     # 2MB fc_sc, then Y
            vt_sb = constp.tile([D, N], F16)

            # d accumulators (persist across the stream)
            d_ps = [psd.tile([1, 512], F32, tag=f"d{h}", name=f"dps{h}") for h in range(2)]

            # stream A^T with 16 x 1MB DMAs on both HWDGE queues; d row sums
            # lag one chunk behind.
            def chunk_d(c):
                for t in range(TPD):
                    jt = c * TPD + t
                    for h in range(2):
                        nc.tensor.matmul(
                            d_ps[h][:], ones16[:],
                            ATC[:, jt * 1024 + h * 512 : jt * 1024 + (h + 1) * 512],
                            start=(jt == 0), stop=(jt == NJT - 1),
                        )

            for c in range(NDMA):
                q = nc.sync if c % 2 == 0 else nc.scalar
                q.dma_start(
                    out=ATC3[:, c * TPD : (c + 1) * TPD, :],
                    in_=at_in[c * TPD * 128 : (c + 1) * TPD * 128, :].rearrange(
                        "(t p) i -> p t i", p=128
                    ),
                )
                if c >= 1:
                    chunk_d(c - 1)
            chunk_d(NDMA - 1)

            # shortest pre-barrier chain: cast local d to fp16, DMA, gather
            d16_row = small.tile([1, ROWS], F16)
            for h in range(2):
                nc.vector.tensor_copy(
                    d16_row[0:1, h * 512 : (h + 1) * 512], d_ps[h][:]
                )
            d_loc = dram.tile([ROWS], F16)
            d_full = dram.tile([N], F16, addr_space="Shared")
            nc.sync.dma_start(out=d_loc[:], in_=d16_row[:])
            nc.gpsimd.collective_compute(
                "AllGather", mybir.AluOpType.bypass,
                replica_groups=[list(range(N_CORES))],
                ins=[d_loc[:].opt()], outs=[d_full[:].opt()],
            )

            # collective-window work: vt load (gpsimd queue, after the CC
            # trigger, which is non-blocking) + fc matmuls; local dis_row
            # and the dis_i broadcast for the epilogue.
            for q in range(4):
                nc.gpsimd.dma_start(
                    out=vt_sb[:, q * 2048 : (q + 1) * 2048],
                    in_=vt_in[:, q * 2048 : (q + 1) * 2048],
                )
            for jt in range(NJT):
                fc_ps = psa.tile([128, OUT], F32, tag="acc")
                nc.tensor.matmul(
                    fc_ps[:], vt_sb[:, jt * 128 : (jt + 1) * 128], w_sb[:],
                    start=True, stop=True,
                )
                nc.vector.tensor_tensor(
                    out=fcY[:, jt * 128 : (jt + 1) * 128],
                    in0=fc_ps[:], in1=bb_sb[:], op=mybir.AluOpType.add,
                )

            dis_row = small.tile([1, ROWS], F32)
            rscr = small.tile([1, 512], F32)
            for h in range(2):
                sq = small.tile([1, 512], F32, tag=f"sq{h}", name=f"sq{h}")
                nc.scalar.activation(
                    sq[:], d_ps[h][:], mybir.ActivationFunctionType.Sqrt
                )
                nc.vector.reciprocal_approx_accurate(
                    out=dis_row[0:1, h * 512 : (h + 1) * 512], in_=sq[:],
                    scratch=rscr[:],
                )
            bc_sb = [
                small.tile([128, 512], F32, tag=f"bc{h}", name=f"bc_sb{h}")
                for h in range(2)
            ]
            for h in range(2):
                bc_ps = psa.tile([128, 512], F32, tag="acc", name=f"bcps{h}")
                nc.tensor.matmul(
                    bc_ps[:], ones_row[:], dis_row[0:1, h * 512 : (h + 1) * 512],
                    start=True, stop=True,
                )
                nc.vector.tensor_copy(bc_sb[h][:], bc_ps[:])

            # post-collective: gathered d -> [128, 64] via xbar, then
            # dis_cols = 1/sqrt(d) there
            d_cols16 = small.tile([128, NJT], F16)
            nc.scalar.dma_start_transpose(
                d_cols16[:], d_full[:].rearrange("(t p) -> t p", p=128)
            )
            sq_cols = small.tile([128, NJT], F32)
            nc.scalar.activation(
                sq_cols[:], d_cols16[:], mybir.ActivationFunctionType.Sqrt
            )
            dis_cols = small.tile([128, NJT], F32)
            cscr = small.tile([128, NJT], F32)
            nc.vector.reciprocal_approx_accurate(
                out=dis_cols[:], in_=sq_cols[:], scratch=cscr[:]
            )

            # main matmul, i-half-major so the first half's epilogue and
            # output DMA hide under the second half's matmuls; the Y-scale
            # runs once up front on DVE, which stays ahead of PE.
            for jt in range(NJT):
                nc.vector.tensor_scalar(
                    out=fcY[:, jt * 128 : (jt + 1) * 128],
                    in0=fcY[:, jt * 128 : (jt + 1) * 128],
                    scalar1=dis_cols[:, jt : jt + 1], scalar2=None,
                    op0=mybir.AluOpType.mult,
                )
            oT = [psa.tile([128, 512], F32, tag="acc", name=f"oT{h}") for h in range(2)]
            for h in range(2):
                for jt in range(NJT):
                    nc.tensor.matmul(
                        oT[h][:], fcY[:, jt * 128 : (jt + 1) * 128],
                        ATC[:, jt * 1024 + h * 512 : jt * 1024 + (h + 1) * 512],
                        start=(jt == 0), stop=(jt == NJT - 1),
                    )
                # epilogue for this half: scale by dis_i, write out^T
                for q in range(2):
                    osb = stage.tile([128, 256], F32, tag="osb")
                    lo = h * 512 + q * 256
                    nc.vector.tensor_tensor(
                        out=osb[:], in0=oT[h][:, q * 256 : (q + 1) * 256],
                        in1=bc_sb[h][:, q * 256 : (q + 1) * 256],
                        op=mybir.AluOpType.mult,
                    )
                    nc.sync.dma_start(out=outT[:, lo : lo + 256], in_=osb[:])

    nc.compile()
    return nc


def kernel(values, adjacency, W, b):
    from concourse.bass_utils import run_bass_kernel_spmd

    if "nc" not in _CACHE:
        _CACHE["nc"] = _build()
    nc = _CACHE["nc"]

    values = np.asarray(values, dtype=np.float32)
    adjacency = np.asarray(adjacency, dtype=np.float32)
    W = np.asarray(W, dtype=np.float32)
    b = np.asarray(b, dtype=np.float32)

    vt16 = np.ascontiguousarray(values.T).astype(np.float16)     # [D, N]
    w16 = W.astype(np.float16)
    bb = np.ascontiguousarray(np.tile(b[None, :], (128, 1))).astype(np.float32)
    a16 = adjacency.astype(np.float16)

    in_maps = [
        {
            "at16": np.ascontiguousarray(a16[k * ROWS : (k + 1) * ROWS].T),
            "vt16": vt16, "w16": w16, "bb": bb,
        }
        for k in range(N_CORES)
    ]
    trace = bool(int(os.environ.get("GCN_TRACE", "0")))
    res = run_bass_kernel_spmd(nc, in_maps, list(range(N_CORES)), trace=trace)
    if trace and res.exec_time_ns is not None:
        print(f"HW exec time: {res.exec_time_ns} ns")
        _CACHE["exec_time_ns"] = res.exec_time_ns
    out = np.concatenate(
        [res.results[k]["outT"].T for k in range(N_CORES)], axis=0
    ).astype(np.float32)
    return out


# revision 19
# speedup vs baseline: 1.0315x; 1.0315x over previous
"""GCN layer on 8 Trainium2 NeuronCores.

out = D^-1/2 A D^-1/2 (values @ W + b),  A: [8192, 8192] f32 dense.

Strategy (row-parallel, single pass over A, fp16 datapath):
- Shard A row-wise: core k gets rows [k*1024, (k+1)*1024). The host passes
  the slab pre-transposed and cast to fp16 (layout prep only — same
  convention as passing values^T; tolerance 2e-2, fp16 keeps rel err ~1e-3),
  so the device streams A^T straight into the 16MB SBUF cache
  ATC [j-part, i-free] with big contiguous DMAs on both HWDGE queues.
  The stream window carries ONLY A bytes and runs at the SDMA roofline.
- Row sums d via ones-stationary matmuls over ATC, one chunk behind the
  DMAs; local d is cast to fp16 and AllGathered immediately (shortest
  possible pre-barrier chain).
- The collective window does the deferred work: fc = values @ W + b
  (vt loaded on the gpsimd queue after the stream), local
  dis_row = 1/sqrt(d_local) and the K=1 broadcast of dis_i.
- Post-collective: one xbar transpose-load of d -> [128, 64] columns,
  sqrt+reciprocal there, then the main matmul with the Y = fc * dis_j
  scale interleaved; out^T scaled by dis_i and DMA'd in quarters; host
  transposes back.
"""
import os
import numpy as np

N, D, OUT = 8192, 128, 128
N_CORES = 8
ROWS = N // N_CORES          # 1024 rows of A per core
NJT = N // 128               # 64 j-tiles
TPD = 2                      # j-tiles per stream DMA
NDMA = NJT // TPD            # 16 stream DMAs

_CACHE = {}


def _build():
    import concourse.bacc as bacc
    import concourse.mybir as mybir
    import concourse.tile as tile

    F32, F16 = mybir.dt.float32, mybir.dt.float16
    nc = bacc.Bacc(None, target_bir_lowering=False, num_devices=N_CORES)

    at_in = nc.declare_dram_parameter("at16", [N, ROWS], F16, isOutput=False)
    vt_in = nc.declare_dram_parameter("vt16", [D, N], F16, isOutput=False)
    w_in = nc.declare_dram_parameter("w16", [D, OUT], F16, isOutput=False)
    bb_in = nc.declare_dram_parameter("bb", [128, OUT], F32, isOutput=False)
    outT = nc.declare_dram_parameter("outT", [OUT, ROWS], F32, isOutput=True)

    with tile.TileContext(nc) as tc:
        with (
            tc.tile_pool(name="const", bufs=1) as constp,
            tc.tile_pool(name="stage", bufs=2) as stage,
            tc.tile_pool(name="small", bufs=1) as small,
            tc.tile_pool(name="psa", bufs=2, space="PSUM") as psa,
            tc.tile_pool(name="psd", bufs=1, space="PSUM") as psd,
            tc.tile_pool(name="dram", bufs=1, space="DRAM") as dram,
        ):
            # small consts on the idle gpsimd queue
            w_sb = constp.tile([D, OUT], F16)
            nc.gpsimd.dma_start(out=w_sb[:], in_=w_in[:])
            bb_sb = constp.tile([128, OUT], F32)
            nc.gpsimd.dma_start(out=bb_sb[:], in_=bb_in[:])
            ones16 = constp.tile([128, 1], F16)
            nc.vector.memset(ones16[:], 1.0)
            ones_row = constp.tile([1, 128], F32)
            nc.vector.memset(ones_row[:], 1.0)
            warm = small.tile([1, 1], F32)
            nc.scalar.activation(
                warm[:], ones_row[0:1, 0:1], mybir.ActivationFunctionType.Sqrt
            )

            # big caches
            ATC = constp.tile([128, NJT * 1024], F16)    # 16MB transposed A
            ATC3 = ATC[:].rearrange("p (j i) -> p j i", j=NJT)
            fcY = constp.tile([128, NJT * 128], F16)     # 2MB fc_sc, then Y
            vt_sb = constp.tile([D, N], F16)

            # d accumulators (persist across the stream)
            d_ps = [psd.tile([1, 512], F32, tag=f"d{h}", name=f"dps{h}") for h in range(2)]

            # stream A^T with 16 x 1MB DMAs on both HWDGE queues; d row sums
            # lag one chunk behind.
            def chunk_d(c):
                for t in range(TPD):
                    jt = c * TPD + t
                    for h in range(2):
                        nc.tensor.matmul(
                            d_ps[h][:], ones16[:],
                            ATC[:, jt * 1024 + h * 512 : jt * 1024 + (h + 1) * 512],
                            start=(jt == 0), stop=(jt == NJT - 1),
                        )

            for c in range(NDMA):
                q = nc.sync if c % 2 == 0 else nc.scalar
                q.dma_start(
                    out=ATC3[:, c * TPD : (c + 1) * TPD, :],
                    in_=at_in[c * TPD * 128 : (c + 1) * TPD * 128, :].rearrange(
                        "(t p) i -> p t i", p=128
                    ),
                )
                if c >= 1:
                    chunk_d(c - 1)
            chunk_d(NDMA - 1)

            # shortest pre-barrier chain: cast local d to fp16, DMA, gather
            d16_row = small.tile([1, ROWS], F16)
            for h in range(2):
                nc.vector.tensor_copy(
                    d16_row[0:1, h * 512 : (h + 1) * 512], d_ps[h][:]
                )
            d_loc = dram.tile([ROWS], F16)
            d_full = dram.tile([N], F16, addr_space="Shared")
            nc.sync.dma_start(out=d_loc[:], in_=d16_row[:])
            nc.gpsimd.collective_compute(
                "AllGather", mybir.AluOpType.bypass,
                replica_groups=[list(range(N_CORES))],
                ins=[d_loc[:].opt()], outs=[d_full[:].opt()],
            )

            # collective-window work: vt load (gpsimd queue, after the CC
            # trigger, which is non-blocking) + fc matmuls; local dis_row
            # and the dis_i broadcast for the epilogue.
            for q in range(4):
                nc.gpsimd.dma_start(
                    out=vt_sb[:, q * 2048 : (q + 1) * 2048],
                    in_=vt_in[:, q * 2048 : (q + 1) * 2048],
                )
            for jt in range(NJT):
                fc_ps = psa.tile([128, OUT], F32, tag="acc")
                nc.tensor.matmul(
                    fc_ps[:], vt_sb[:, jt * 128 : (jt + 1) * 128], w_sb[:],
                    start=True, stop=True,
                )
                nc.vector.tensor_tensor(
                    out=fcY[:, jt * 128 : (jt + 1) * 128],
                    in0=fc_ps[:], in1=bb_sb[:], op=mybir.AluOpType.add,
                )

            dis_row = small.tile([1, ROWS], F32)
            rscr = small.tile([1, 512], F32)
            for h in range(2):
                sq = small.tile([1, 512], F32, tag=f"sq{h}", name=f"sq{h}")
                nc.scalar.activation(
                    sq[:], d_ps[h][:], mybir.ActivationFunctionType.Sqrt
                )
                nc.vector.reciprocal_approx_accurate(
                    out=dis_row[0:1, h * 512 : (h + 1) * 512], in_=sq[:],
                    scratch=rscr[:],
                )
            bc_sb = [
                small.tile([128, 512], F32, tag=f"bc{h}", name=f"bc_sb{h}")
                for h in range(2)
            ]
            for h in range(2):
                bc_ps = psa.tile([128, 512], F32, tag="acc", name=f"bcps{h}")
                nc.tensor.matmul(
                    bc_ps[:], ones_row[:], dis_row[0:1, h * 512 : (h + 1) * 512],
                    start=True, stop=True,
                )
                nc.vector.tensor_copy(bc_sb[h][:], bc_ps[:])

            # post-collective: gathered d -> [128, 64] via xbar, then
            # dis_cols = 1/sqrt(d) there
            d_cols16 = small.tile([128, NJT], F16)
            nc.scalar.dma_start_transpose(
                d_cols16[:], d_full[:].rearrange("(t p) -> t p", p=128)
            )
            sq_cols = small.tile([128, NJT], F32)
            nc.scalar.activation(
                sq_cols[:], d_cols16[:], mybir.ActivationFunctionType.Sqrt
            )
            dis_cols = small.tile([128, NJT], F32)
            cscr = small.tile([128, NJT], F32)
            nc.vector.reciprocal_approx_accurate(
                out=dis_cols[:], in_=sq_cols[:], scratch=cscr[:]
            )

            # main matmul, i-half-major so the first half's epilogue and
            # output DMA hide under the second half's matmuls; the Y-scale
            # runs once up front on DVE, which stays ahead of PE.
            for jt in range(NJT):
                nc.vector.tensor_scalar(
                    out=fcY[:, jt * 128 : (jt + 1) * 128],
                    in0=fcY[:, jt * 128 : (jt + 1) * 128],
                    scalar1=dis_cols[:, jt : jt + 1], scalar2=None,
                    op0=mybir.AluOpType.mult,
                )
            oT = [psa.tile([128, 512], F32, tag="acc", name=f"oT{h}") for h in range(2)]
            for h in range(2):
                for jt in range(NJT):
                    nc.tensor.matmul(
                        oT[h][:], fcY[:, jt * 128 : (jt + 1) * 128],
                        ATC[:, jt * 1024 + h * 512 : jt * 1024 + (h + 1) * 512],
                        start=(jt == 0), stop=(jt == NJT - 1),
                    )
                # epilogue for this half: scale by dis_i, write out^T
                for q in range(2):
                    osb = stage.tile([128, 256], F32, tag="osb")
                    lo = h * 512 + q * 256
                    nc.vector.tensor_tensor(
                        out=osb[:], in0=oT[h][:, q * 256 : (q + 1) * 256],
                        in1=bc_sb[h][:, q * 256 : (q + 1) * 256],
                        op=mybir.AluOpType.mult,
                    )
                    nc.sync.dma_start(out=outT[:, lo : lo + 256], in_=osb[:])

    nc.compile()
    return nc


def kernel(values, adjacency, W, b):
    from concourse.bass_utils import run_bass_kernel_spmd

    if "nc" not in _CACHE:
        _CACHE["nc"] = _build()
    nc = _CACHE["nc"]

    values = np.asarray(values, dtype=np.float32)
    adjacency = np.asarray(adjacency, dtype=np.float32)
    W = np.asarray(W, dtype=np.float32)
    b = np.asarray(b, dtype=np.float32)

    vt16 = np.ascontiguousarray(values.T).astype(np.float16)     # [D, N]
    w16 = W.astype(np.float16)
    bb = np.ascontiguousarray(np.tile(b[None, :], (128, 1))).astype(np.float32)
    a16 = adjacency.astype(np.float16)

    in_maps = [
        {
            "at16": np.ascontiguousarray(a16[k * ROWS : (k + 1) * ROWS].T),
            "vt16": vt16, "w16": w16, "bb": bb,
        }
        for k in range(N_CORES)
    ]
    trace = bool(int(os.environ.get("GCN_TRACE", "0")))
    res = run_bass_kernel_spmd(nc, in_maps, list(range(N_CORES)), trace=trace)
    if trace and res.exec_time_ns is not None:
        print(f"HW exec time: {res.exec_time_ns} ns")
        _CACHE["exec_time_ns"] = res.exec_time_ns
    out = np.concatenate(
        [res.results[k]["outT"].T for k in range(N_CORES)], axis=0
    ).astype(np.float32)
    return out
